# revision 11
# baseline (speedup 1.0000x reference)
"""PointContrastive loss on 8 Trainium2 NeuronCores.

Sharding: data-parallel over objects (2 objects/core). Each core computes its
64 masks' summed/avg features via bf16 segment-matmuls (mask values and the
appended ones-columns are exact in bf16; accumulation is fp32 in PSUM), then
its local logit slab C^T = A_loc @ E_all^T [64 local j, 512 global i] from
purely local data. From that slab it derives:
  - pts_loss ingredients (row max + sum-exp over all 512 columns)
  - texts-direction partials (per global row i: max and shifted sum-exp over
    the local 64 columns), via 4 PE transposes of the slab.
Per-core device output is ~4.5 KB; the host does the 8-way log-sum-exp
combine and the two nonzero-means (pure O(N) postprocessing on 8 KB).
There is no cross-core communication on the device, so cores run fully
independently.

Device input layouts are chosen so every DMA is one long contiguous
descriptor per partition:
 - maskT: [128, 64*64] bf16  block-diagonal transposed masks (chunk-major)
 - pts:   [128, 64*770] bf16 partition-major point features: element
          [p, c*770 + m] = net_out[core_slab + c*128 + p, m], with columns
          768/769 of each chunk = 1.0 (npts falls out of the same matmul)
 - embsT: [768, 512] f32r    mask_embs transposed (same on all cores)
 - eloc:  [64, 768] f32      this core's row slice of mask_embs
"""

import numpy as np
import ml_dtypes
import concourse.bass as bass
from concourse import bacc
import concourse.mybir as mybir
import concourse.tile as tile
from concourse.bass_utils import run_bass_kernel_spmd
from concourse.masks import make_identity

BS, P, M, D = 16, 4096, 32, 768
NCORES = 8
OBJ = BS // NCORES          # objects per core = 2
NM = OBJ * M                # local masks = 64
N = BS * M                  # global masks = 512
KPO = P // 128              # k-chunks per object = 32
NCH = OBJ * KPO             # k-chunks per core = 64
DCH = D // 128              # feature chunks = 6
DP = D + 2                  # pts row width incl. ones columns = 770
ITS = N // 128              # row tiles of the global mask axis = 4
GRP = 4                     # k-chunks per pts DMA
NGRP = NCH // GRP           # pts DMAs = 16

f32 = mybir.dt.float32
f32r = mybir.dt.float32r
bf16 = mybir.dt.bfloat16

_nc_cache = None


def _build():
    nc = bacc.Bacc("TRN2", target_bir_lowering=False, debug=False, num_devices=NCORES)
    maskT_d = nc.dram_tensor("maskT", [128, NCH * NM], bf16, kind="ExternalInput").ap()
    pts_d = nc.dram_tensor("pts", [128, NCH * DP], bf16, kind="ExternalInput").ap()
    embsT_d = nc.dram_tensor("embsT", [D, N], bf16, kind="ExternalInput").ap()
    eloc_d = nc.dram_tensor("eloc", [NM, D], f32, kind="ExternalInput").ap()
    sca_d = nc.dram_tensor("sca", [128, 1], f32, kind="ExternalInput").ap()
    out_d = nc.dram_tensor("out", [NM, 4], f32, kind="ExternalOutput").ap()
    st_d = nc.dram_tensor("stats", [128, 2 * ITS], f32, kind="ExternalOutput").ap()

    with tile.TileContext(nc) as tc:
        with (
            tc.tile_pool(name="sb", bufs=1) as sb,
            tc.tile_pool(name="pts_pool", bufs=4) as pts_pool,
            tc.tile_pool(name="ps", bufs=1, space="PSUM") as ps,
            tc.tile_pool(name="pst", bufs=2, space="PSUM") as pst,
        ):
            # ---- persistent loads (ACT HWDGE queue; pts stream owns Sync) ----
            ident = sb.tile([128, 128], f32)
            make_identity(nc, ident[:])
            sca = sb.tile([128, 1], f32)
            nc.scalar.dma_start(sca[:], sca_d[:])
            mth = NCH * NM // 2
            mt0 = sb.tile([128, mth], bf16)
            nc.scalar.dma_start(mt0[:], maskT_d[:, 0:mth])
            mt1 = sb.tile([128, mth], bf16)
            nc.scalar.dma_start(mt1[:], maskT_d[:, mth:])
            el = sb.tile([NM, D], f32)
            nc.scalar.dma_start(el[:], eloc_d[:])
            et = []
            for j in range(DCH):
                e1 = sb.tile([128, N], bf16, name=f"et{j}")
                nc.scalar.dma_start(e1[:], embsT_d[j * 128:(j + 1) * 128, :])
                et.append(e1)

            # ---- phase 1: segment sums over 64 k-chunks, 4 chunks per DMA ----
            s_ps = ps.tile([NM, DP], f32)
            for g in range(NGRP):
                pt = pts_pool.tile([128, GRP * DP], bf16, name="pt")
                nc.sync.dma_start(pt[:], pts_d[:, g * GRP * DP:(g + 1) * GRP * DP])
                for q in range(GRP):
                    c = GRP * g + q
                    mtx, co = (mt0, c) if c < NCH // 2 else (mt1, c - NCH // 2)
                    lhs = mtx[:, co * NM:(co + 1) * NM]
                    st, sp = (c == 0), (c == NCH - 1)
                    nc.tensor.matmul(s_ps[:, 0:512], lhs,
                                     pt[:, q * DP:q * DP + 512], start=st, stop=sp)
                    nc.tensor.matmul(s_ps[:, 512:DP], lhs,
                                     pt[:, q * DP + 512:(q + 1) * DP], start=st, stop=sp)

            npts = sb.tile([NM, 1], f32)
            nc.vector.tensor_copy(npts[:], s_ps[:, D:D + 1])
            t1 = sb.tile([NM, 1], f32)
            nc.vector.tensor_scalar_add(t1[:], npts[:], 1e-12)
            inv = sb.tile([NM, 1], f32)
            nc.vector.reciprocal(inv[:], t1[:])
            a = sb.tile([NM, D], f32)
            nc.vector.tensor_scalar_mul(a[:], s_ps[:, 0:D], inv[:])

            # ---- transpose A locally (A^T chunks feed the C matmuls) ----
            at = []
            for j in range(DCH):
                tp = pst.tile([128, NM], f32, name="tp")
                nc.tensor.transpose(tp[:], a[:, j * 128:(j + 1) * 128],
                                    ident[0:NM, 0:NM])
                a1 = sb.tile([128, NM], bf16, name=f"at{j}")
                nc.vector.tensor_copy(a1[:], tp[:])
                at.append(a1)

            # ---- phase 2 (all local): C^T[j_loc, i] = A_loc[j_loc] . E_all[i]
            c_ps = ps.tile([NM, N], f32)
            for j in range(DCH):
                nc.tensor.matmul(c_ps[:], at[j][:], et[j][:],
                                 start=(j == 0), stop=(j == DCH - 1))

            out_sb = sb.tile([NM, 4], f32)
            # pts_loss ingredients: row max + sum-exp of s*(C - mC)
            mC = sb.tile([NM, 1], f32)
            nc.vector.reduce_max(mC[:], c_ps[:], axis=mybir.AxisListType.X)
            negsmC = sb.tile([NM, 1], f32)
            nc.vector.tensor_mul(negsmC[:], mC[:], sca[0:NM, :])
            nc.vector.tensor_scalar_mul(negsmC[:], negsmC[:], -1.0)
            exC = sb.tile([NM, N], f32)
            seC = sb.tile([NM, 1], f32)
            nc.scalar.activation(exC[:], c_ps[:], mybir.ActivationFunctionType.Exp,
                                 bias=negsmC[:], scale=sca[0:NM, :], accum_out=seC[:])
            # raw C^T in SBUF for the texts-direction transposes
            csb = sb.tile([NM, N], f32)
            st8 = sb.tile([128, 2 * ITS], f32)
            negm = sb.tile([128, ITS], f32)
            for it in range(ITS):
                nc.vector.tensor_copy(csb[:, it * 128:(it + 1) * 128],
                                      c_ps[:, it * 128:(it + 1) * 128])
                g1 = pst.tile([128, NM], f32, name="gt")
                nc.tensor.transpose(g1[:], csb[:, it * 128:(it + 1) * 128],
                                    ident[0:NM, 0:NM])
                nc.vector.reduce_max(st8[:, it:it + 1], g1[:],
                                     axis=mybir.AxisListType.X)
                nc.vector.tensor_mul(negm[:, it:it + 1], st8[:, it:it + 1], sca[:])
                nc.vector.tensor_scalar_mul(negm[:, it:it + 1], negm[:, it:it + 1],
                                            -1.0)
                ex4 = sb.tile([128, NM], f32, name="ex4")
                nc.scalar.activation(ex4[:], g1[:], mybir.ActivationFunctionType.Exp,
                                     bias=negm[:, it:it + 1], scale=sca[:],
                                     accum_out=st8[:, ITS + it:ITS + it + 1])
            nc.sync.dma_start(st_d[:], st8[:])

            # diag[i] = eloc[i] . a[i]  (off the critical path)
            dtmp = sb.tile([NM, D], f32)
            nc.vector.tensor_mul(dtmp[:], a[:], el[:])
            diag = sb.tile([NM, 1], f32)
            nc.vector.reduce_sum(diag[:], dtmp[:], axis=mybir.AxisListType.X)

            # out: col0 = diag, col1 = mC, col2 = seC, col3 = npts
            nc.vector.tensor_copy(out_sb[:, 0:1], diag[:])
            nc.vector.tensor_copy(out_sb[:, 1:2], mC[:])
            nc.vector.tensor_copy(out_sb[:, 2:3], seC[:])
            nc.vector.tensor_copy(out_sb[:, 3:4], npts[:])
            nc.sync.dma_start(out_d[:], out_sb[:])
    nc.compile()
    return nc


def _prep_inputs(net_out, mask_embs, mask_pts, logit_scale):
    net_out = np.asarray(net_out, dtype=np.float32)
    mask_embs = np.ascontiguousarray(np.asarray(mask_embs, dtype=np.float32))
    mask_pts = np.asarray(mask_pts, dtype=np.float32)
    s = float(np.exp(np.float64(np.asarray(logit_scale).reshape(-1)[0])))

    # partition-major pts with ones-columns: [core, p, c, m]
    pts_all = np.ones((BS * P, DP), dtype=ml_dtypes.bfloat16)
    pts_all[:, 0:D] = net_out.astype(ml_dtypes.bfloat16)
    # [core, c, p, m] -> [core, p, c, m]
    pts_pm = pts_all.reshape(NCORES, NCH, 128, DP).transpose(0, 2, 1, 3)

    # block-diagonal transposed masks: h[c, p, b*KPO+k, b*M+m] = mask[2c+b, m, k*128+p]
    v = mask_pts.reshape(NCORES, OBJ, M, KPO, 128).transpose(0, 1, 4, 3, 2)
    h = np.zeros((NCORES, 128, NCH, NM), dtype=ml_dtypes.bfloat16)
    for b in range(OBJ):
        h[:, :, b * KPO:(b + 1) * KPO, b * M:(b + 1) * M] = v[:, b]
    embsT = np.ascontiguousarray(mask_embs.T.astype(ml_dtypes.bfloat16))
    sca = np.full((128, 1), s, dtype=np.float32)

    in_maps = []
    for c in range(NCORES):
        in_maps.append({
            "maskT": np.ascontiguousarray(h[c].reshape(128, NCH * NM)),
            "pts": np.ascontiguousarray(pts_pm[c].reshape(128, NCH * DP)),
            "embsT": embsT,
            "eloc": np.ascontiguousarray(mask_embs[c * NM:(c + 1) * NM, :]),
            "sca": sca,
        })
    return in_maps, s


def _nonzero_mean(x):
    nz = x > 0
    cnt = int(nz.sum())
    if cnt == 0:
        return np.float32(0.0)
    return np.where(nz, x, 0.0).sum(dtype=np.float64) / cnt


def _combine(results, s):
    outs = [np.asarray(results[c]["out"]) for c in range(NCORES)]
    diag = np.concatenate([o[:, 0] for o in outs]).astype(np.float64)   # [512]
    mC = np.concatenate([o[:, 1] for o in outs]).astype(np.float64)     # raw row max
    seC = np.concatenate([o[:, 2] for o in outs]).astype(np.float64)    # sum exp(s*(C-mC))
    npts = np.concatenate([o[:, 3] for o in outs])
    # texts-direction partials: stats[p, it] (raw max), stats[p, ITS+it] (sumexp)
    m_r = np.stack([np.asarray(results[c]["stats"])[:, 0:ITS].T.reshape(N)
                    for c in range(NCORES)]).astype(np.float64)         # [8, 512]
    s_r = np.stack([np.asarray(results[c]["stats"])[:, ITS:2 * ITS].T.reshape(N)
                    for c in range(NCORES)]).astype(np.float64)
    Mx = m_r.max(axis=0)
    T = (s_r * np.exp(s * (m_r - Mx))).sum(axis=0)
    lse_rows = np.log(T) + s * Mx
    texts = lse_rows - s * diag
    ptsl = np.log(seC) + s * mC - s * diag
    valid = npts > 0
    texts = np.where(valid, texts, 0.0)
    ptsl = np.where(valid, ptsl, 0.0)
    return np.asarray(
        (_nonzero_mean(texts) + _nonzero_mean(ptsl)) / 2.0, dtype=np.float32)


def _run(trace=False, **inputs):
    global _nc_cache
    if _nc_cache is None:
        _nc_cache = _build()
    in_maps, s = _prep_inputs(
        inputs["net_out"], inputs["mask_embs"], inputs["mask_pts"],
        inputs["logit_scale"])
    res = run_bass_kernel_spmd(
        _nc_cache, in_maps, core_ids=list(range(NCORES)), trace=trace)
    return _combine(res.results, s), res


def kernel(**inputs) -> np.ndarray:
    out, _ = _run(trace=False, **inputs)
    return out
